# revision 40
# baseline (speedup 1.0000x reference)
# Bayesian SkipGram loss on 8 Trainium2 NeuronCores (Bass/Tile, SPMD).
#
# Sharding: data-parallel over batch B=1024 for the encoder / KL / rec-dot
# (BS=128 rows per core), AllGather of the latent zT across cores, then
# vocab-parallel (V split 8 ways) z @ W_vocab with log-softmax denominators
# accumulated on-the-fly. Per-core partial exp-sums / KL / rec dots are
# combined on the host (tiny [B]-sized math).
#
# Layout: everything transposed ([feature, batch], feature on SBUF
# partitions) so the PE matmul out = lhsT.T @ rhs needs no transposes.
#
# Optimizations over the naive structure:
#  - inputs packed into 4 host-prepared buffers -> 4+4 DMA issues instead
#    of ~29 (DMA issue dispatch is ~0.5us each and serializes the prologue)
#  - the AllGather is issued as soon as zT exists; each core then processes
#    its OWN z block first (straight from SBUF) plus the whole KL/rec
#    epilogue while the collective is in flight
#  - remote z blocks are fetched with a dynamically-indexed DMA
#    (slot = (my_core + j) % 8), so every core starts with its local block
#    without needing per-core program variants; the host un-rotates columns
#  - the exp row-sum work is split across two engines: ScalarE does cols
#    [0:4464] of each block with exact table exp + hardware accumulation,
#    VectorE does cols [4464:6288] with a Schraudolph-style exp
#    (int32(A*x+B) bitcast to f32) + reduce. Both engines run ~fully busy
#    in steady state, cutting the ScalarE-only exp time by ~27%.
#  - encoder inputs/weights travel in bf16 (half the DMA bytes), softplus
#    is fused into single activations via the bias port (ln(x+1)), and a
#    couple of dummy matmuls keep the PE p-state ramp alive while the
#    input DMAs land.
import os
import sys

import numpy as np

for _p in ("/opt/trn_rl_repo", "/root/.axon_site/_ro/trn_rl_repo"):
    if os.path.isdir(_p) and _p not in sys.path:
        sys.path.insert(0, _p)

V = 50257
D = 128
B = 1024
C = 10
NCORES = 8
BS = B // NCORES          # batch rows per core
VS = 6288                 # vocab columns per core (ceil(V/8) -> x16)
NPAD = NCORES * VS - V    # zero-padded W columns on the last core

ACT_CHUNKS = [(0, 1536), (1536, 1536), (3072, 1392)]
DVE_CHUNKS = [(4464, 512), (4976, 512), (5488, 512), (6000, 288)]

# Schraudolph exp: exp(x) ~= bitcast_f32(int32(A*x + B)).  B is shifted by
# 486411 to null the mean relative error of the mantissa interpolation, so
# row-sums of ~6k terms are nearly unbiased.
A_EXP = float(np.float32(2 ** 23 / np.log(2.0)))
B_EXP = float(np.float32(1065353216 - 486411))
# exp(0) under the approximation (affine result is an exact int in f32)
PAD_EXPVAL = float(
    np.int32(int(np.float32(B_EXP))).view(np.float32))

_STATE = {}

LAST_EXEC_TIME_NS = None
LAST_RESULTS = None


def _patch_act_tables():
    import concourse.bacc as bacc_mod
    import concourse.mybir as mybir
    if getattr(bacc_mod, "_bsg_act_patch", False):
        return
    AF = mybir.ActivationFunctionType
    orig = bacc_mod.get_activation_tables

    def patched(module_arch):
        tabs = orig(module_arch)
        both = [n for n, fns in tabs.items()
                if AF.Exp in fns and AF.Ln in fns and AF.Relu in fns]
        if both:
            keep = both[0]
            for n, fns in tabs.items():
                if n != keep:
                    fns.discard(AF.Exp)
                    fns.discard(AF.Ln)
                    fns.discard(AF.Relu)
        return tabs

    bacc_mod.get_activation_tables = patched
    bacc_mod._bsg_act_patch = True


def _build_module(repeat=1, skip_cc=False, skip_big=False):
    import concourse.mybir as mybir
    import concourse.tile as tile
    from concourse import bacc
    _patch_act_tables()

    f32 = mybir.dt.float32
    i32 = mybir.dt.int32
    bf16 = mybir.dt.bfloat16
    AF = mybir.ActivationFunctionType
    ALU = mybir.AluOpType
    X = mybir.AxisListType.X

    nc = bacc.Bacc("TRN2", target_bir_lowering=False, debug=False,
                   num_devices=NCORES, num_swdge_queues=4)

    # packed inputs (see prepare_in_maps for column layouts)
    wpk = nc.dram_tensor("wpk", [D, 8 * D], bf16, kind="ExternalInput")
    cxp = nc.dram_tensor("cxp", [D, (1 + C) * BS], bf16, kind="ExternalInput")
    pps = nc.dram_tensor("pps", [D, 3 * BS + 5], f32, kind="ExternalInput")
    wsh = nc.dram_tensor("wsh", [D, VS], bf16, kind="ExternalInput")

    o_sum = nc.dram_tensor("o_sum", [BS, NCORES], f32, kind="ExternalOutput")
    o_kl = nc.dram_tensor("o_kl", [1, BS], f32, kind="ExternalOutput")
    o_rec = nc.dram_tensor("o_rec", [1, BS], f32, kind="ExternalOutput")

    with tile.TileContext(nc) as tc:
        with tc.tile_pool(name="const", bufs=1) as cp, \
             tc.tile_pool(name="work", bufs=2) as wp, \
             tc.tile_pool(name="expp", bufs=2) as epool, \
             tc.tile_pool(name="dram", bufs=1, space="DRAM") as dp:
          for _rep in range(repeat):
              # ---- input DMAs (few, large, ordered by first use)
              t_cxp = cp.tile([D, (1 + C) * BS], bf16, tag="cxp")
              nc.scalar.dma_start(out=t_cxp[:, 0:6 * BS],
                                  in_=cxp[:, 0:6 * BS])
              t_wpk = cp.tile([D, 8 * D], bf16, tag="wpk")
              nc.sync.dma_start(out=t_wpk[:], in_=wpk[:])
              t_pps = cp.tile([D, 3 * BS + 5], f32, tag="pps")
              nc.sync.dma_start(out=t_pps[:], in_=pps[:])
              nc.scalar.dma_start(out=t_cxp[:, 6 * BS:11 * BS],
                                  in_=cxp[:, 6 * BS:11 * BS])
              t_wsh = cp.tile([D, VS], bf16, tag="wsh")
              for _i in range(4):
                  _w = VS // 4
                  nc.gpsimd.dma_start(out=t_wsh[:, _i * _w:(_i + 1) * _w],
                                      in_=wsh[:, _i * _w:(_i + 1) * _w])

              t_ce = t_cxp[:, 0:BS]
              cxsl = lambda k: t_cxp[:, (1 + k) * BS:(2 + k) * BS]
              t_we0 = t_wpk[:, 0:2 * D]
              t_we1 = t_wpk[:, 2 * D:4 * D]
              t_wm0 = t_wpk[:, 4 * D:5 * D]
              t_wm1 = t_wpk[:, 5 * D:6 * D]
              t_wv0 = t_wpk[:, 6 * D:7 * D]
              t_wv1 = t_wpk[:, 7 * D:8 * D]
              t_pm = t_pps[:, 0:BS]
              t_pv = t_pps[:, BS:2 * BS]
              t_ws = t_pps[:, 2 * BS:3 * BS]
              t_be0 = t_pps[:, 3 * BS + 0:3 * BS + 1]
              t_be1 = t_pps[:, 3 * BS + 1:3 * BS + 2]
              t_bm = t_pps[:, 3 * BS + 2:3 * BS + 3]
              t_bv = t_pps[:, 3 * BS + 3:3 * BS + 4]
              t_ep = t_pps[:, 3 * BS + 4:3 * BS + 5]

              t_one = cp.tile([D, 1], f32, tag="one")
              nc.vector.memset(t_one[:], 1.0)

              # ---- PE warm-up: dummy matmuls during the input-DMA wait
              # keep the tensor engine's p-state ramp going so the encoder
              # matmuls run at full clock instead of the cold/mid rate
              wrm = cp.tile([D, 512], bf16, tag="wrm")
              nc.vector.memset(wrm[:], 0.0)
              with tc.tile_pool(name="wps", bufs=1, space="PSUM") as wpool:
                  wps = wpool.tile([BS, 512], f32, tag="warm")
                  for _ in range(2):
                      nc.tensor.matmul(wps[:, 0:BS], wrm[:, 0:BS],
                                       wrm[:, 0:BS], start=True, stop=True)

              # ---- encoder: hT[oo] = sum_k relu(W_enc.T @ [ce; cx_k] + b)
              # all 20 matmuls stream into two big PSUM tiles, then one
              # split relu pass (DVE low half, ScalarE high half) per tile
              # and a flat add-tree on DVE.
              hT = []
              with tc.tile_pool(name="hps", bufs=2, space="PSUM") as hp:
                  for oo in range(2):
                      pbig = hp.tile([D, C * BS], f32, tag="pbig")
                      for k in range(C):
                          psl = pbig[:, k * BS:(k + 1) * BS]
                          nc.tensor.matmul(psl, t_we0[:, oo * D:(oo + 1) * D],
                                           t_ce, start=True, stop=False)
                          nc.tensor.matmul(psl, t_we1[:, oo * D:(oo + 1) * D],
                                           cxsl(k), start=False, stop=True)
                      rbig = cp.tile([D, C * BS], bf16, tag=f"rbig{oo}")
                      bias = t_be0 if oo == 0 else t_be1
                      nc.vector.tensor_scalar(
                          rbig[:, 0:5 * BS], pbig[:, 0:5 * BS], bias,
                          0.0, op0=ALU.add, op1=ALU.max)
                      nc.scalar.activation(
                          rbig[:, 5 * BS:10 * BS], pbig[:, 5 * BS:10 * BS],
                          AF.Relu, bias=bias)
                      # tree-sum the 10 slices into slice 0 (fat adds)
                      sl = lambda a, w: rbig[:, a * BS:(a + w) * BS]
                      nc.vector.tensor_tensor(sl(0, 5), sl(0, 5), sl(5, 5),
                                              op=ALU.add)
                      nc.vector.tensor_tensor(sl(0, 2), sl(0, 2), sl(2, 2),
                                              op=ALU.add)
                      nc.vector.tensor_tensor(sl(0, 1), sl(0, 1), sl(1, 1),
                                              op=ALU.add)
                      nc.vector.tensor_tensor(sl(0, 1), sl(0, 1), sl(4, 1),
                                              op=ALU.add)
                      hT.append(rbig)

                  # meanT = (h @ W_mean + b_mean).T
                  p_m = hp.tile([D, BS], f32, tag="mv")
                  nc.tensor.matmul(p_m[:], t_wm0, hT[0][:, 0:BS],
                                   start=True, stop=False)
                  nc.tensor.matmul(p_m[:], t_wm1, hT[1][:, 0:BS],
                                   start=False, stop=True)
                  meanT = cp.tile([D, BS], f32, tag="meanT")
                  nc.vector.tensor_scalar(meanT[:], p_m[:], t_bm, None,
                                          op0=ALU.add)

                  # varT = softplus(h @ W_var + b_var) = ln(1 + exp(x + b))
                  p_v = hp.tile([D, BS], f32, tag="mv")
                  nc.tensor.matmul(p_v[:], t_wv0, hT[0][:, 0:BS],
                                   start=True, stop=False)
                  nc.tensor.matmul(p_v[:], t_wv1, hT[1][:, 0:BS],
                                   start=False, stop=True)
                  sp1 = wp.tile([D, BS], f32, tag="sp1")
                  nc.scalar.activation(sp1[:], p_v[:], AF.Exp, bias=t_bv)
                  varT = cp.tile([D, BS], f32, tag="varT")
                  nc.scalar.activation(varT[:], sp1[:], AF.Ln, bias=1.0)

                  # zTb = bf16(meanT + exp(varT / 2) * eps), fused on DVE
                  ez = wp.tile([D, BS], f32, tag="ez")
                  nc.scalar.activation(ez[:], varT[:], AF.Exp, scale=0.5)
                  zTb = cp.tile([D, BS], bf16, tag="zTb")
                  nc.vector.scalar_tensor_tensor(
                      zTb[:], ez[:], t_ep, meanT[:],
                      op0=ALU.mult, op1=ALU.add)

                  # ---- allgather zT (issued ASAP; big loop starts local)
                  cc_in = dp.tile([D, BS], bf16, tag="ccin")
                  cc_out = dp.tile([NCORES, D, BS], bf16, tag="ccout")
                  nc.sync.dma_start(out=cc_in[:], in_=zTb[:])
                  if not skip_cc:
                      nc.gpsimd.collective_compute(
                          "AllGather", ALU.bypass,
                          replica_groups=[list(range(NCORES))],
                          ins=[cc_in.opt()], outs=[cc_out.opt()])

                  # ---- KL / rec epilogue (overlaps the collective)
                  sp2 = wp.tile([D, BS], f32, tag="sp2")
                  nc.scalar.activation(sp2[:], t_pv, AF.Exp)
                  pvs = wp.tile([D, BS], f32, tag="pvs")
                  nc.scalar.activation(pvs[:], sp2[:], AF.Ln, bias=1.0)
                  rpv = wp.tile([D, BS], f32, tag="rpv")
                  nc.vector.reciprocal(rpv[:], pvs[:])
                  lnpv = wp.tile([D, BS], f32, tag="lnpv")
                  nc.scalar.activation(lnpv[:], pvs[:], AF.Ln)
                  lnvar = wp.tile([D, BS], f32, tag="lnvar")
                  nc.scalar.activation(lnvar[:], varT[:], AF.Ln)

                  diff = wp.tile([D, BS], f32, tag="diff")
                  nc.vector.tensor_tensor(diff[:], t_pm, meanT[:],
                                          op=ALU.subtract)
                  d2 = wp.tile([D, BS], f32, tag="d2")
                  nc.vector.tensor_tensor(d2[:], diff[:], diff[:],
                                          op=ALU.mult)
                  nc.vector.tensor_tensor(d2[:], d2[:], varT[:], op=ALU.add)
                  kacc = wp.tile([D, BS], f32, tag="kacc")
                  nc.vector.tensor_tensor(kacc[:], d2[:], rpv[:],
                                          op=ALU.mult)
                  lnr = wp.tile([D, BS], f32, tag="lnr")
                  nc.vector.scalar_tensor_tensor(
                      lnr[:], lnpv[:], -1.0, lnvar[:],
                      op0=ALU.add, op1=ALU.subtract)
                  nc.vector.tensor_tensor(kacc[:], kacc[:], lnr[:],
                                          op=ALU.add)
                  wz = wp.tile([D, BS], f32, tag="wz")
                  nc.vector.tensor_tensor(wz[:], zTb[:], t_ws, op=ALU.mult)

                  kl_ps = hp.tile([D, BS], f32, tag="mv")
                  nc.tensor.matmul(kl_ps[0:1, :], t_one[:], kacc[:],
                                   start=True, stop=True)
                  kl_sb = wp.tile([1, BS], f32, tag="klsb")
                  nc.vector.tensor_copy(kl_sb[:], kl_ps[0:1, :])
                  nc.sync.dma_start(out=o_kl[:], in_=kl_sb[:])
                  rec_ps = hp.tile([D, BS], f32, tag="mv")
                  nc.tensor.matmul(rec_ps[0:1, :], t_one[:], wz[:],
                                   start=True, stop=True)
                  rec_sb = wp.tile([1, BS], f32, tag="recsb")
                  nc.vector.tensor_copy(rec_sb[:], rec_ps[0:1, :])
                  nc.sync.dma_start(out=o_rec[:], in_=rec_sb[:])
              # hp (encoder/epilogue PSUM) closed: big loop gets all 8 banks

              # ---- vocab-parallel logits + exp row-sums
              # iteration j processes z block (my_core + j) % 8:
              #   j = 0 is the local block (no collective dependency)
              sumexp = cp.tile([BS, NCORES], f32, tag="sumexp")
              accs = []
              pid = nc.sync.partition_id()
              nbig = 0 if skip_big else NCORES
              if skip_big:
                  nc.vector.memset(sumexp[:], 0.0)
              with tc.tile_pool(name="actps", bufs=2, space="PSUM") as ap_, \
                   tc.tile_pool(name="dveps", bufs=2, space="PSUM") as dpp:
                  for j in range(nbig):
                      if j == 0:
                          zt = zTb
                      else:
                          zt = wp.tile([D, BS], bf16, tag="zt")
                          slot = (pid + j) % NCORES
                          nc.sync.dma_start(out=zt[:], in_=cc_out[slot])
                      acc = cp.tile([BS, 7], f32, tag=f"acc{j}")
                      accs.append(acc)

                      def emit_act_chunks():
                          for ci, (off, w) in enumerate(ACT_CHUNKS):
                              p = ap_.tile([BS, 1536], f32, tag="pa")
                              for s in range(0, w, 512):
                                  nc.tensor.matmul(
                                      p[:, s:s + 512], zt[:],
                                      t_wsh[:, off + s:off + s + 512],
                                      start=True, stop=True)
                              e = epool.tile([BS, 1536], bf16, tag="e")
                              nc.scalar.activation(
                                  e[:, 0:w], p[:, 0:w], AF.Exp,
                                  accum_out=acc[:, ci:ci + 1])

                      def emit_dve_chunks():
                          for di, (off, w) in enumerate(DVE_CHUNKS):
                              p = dpp.tile([BS, 512], f32, tag="pd")
                              nc.tensor.matmul(p[:, 0:w], zt[:],
                                               t_wsh[:, off:off + w],
                                               start=True, stop=True)
                              si = epool.tile([BS, 512], i32, tag="si")
                              nc.vector.tensor_scalar(
                                  si[:, 0:w], p[:, 0:w], A_EXP, B_EXP,
                                  op0=ALU.mult, op1=ALU.add)
                              nc.vector.reduce_sum(
                                  acc[:, 3 + di:4 + di],
                                  si[:, 0:w].bitcast(f32), axis=X)

                      # DVE chunks' matmuls go first so the last block's
                      # VectorE stream isn't stuck behind 12 ScalarE-chunk
                      # matmuls; block 1 keeps ScalarE first so the engine
                      # restarts ASAP after the collective wait.
                      if j == 1:
                          emit_act_chunks()
                          emit_dve_chunks()
                      else:
                          emit_dve_chunks()
                          emit_act_chunks()
                      # combine the previous block's accumulators while this
                      # block is still in flight (keeps DVE from stalling on
                      # the Activation engine's last accum readout)
                      if j > 0:
                          nc.vector.reduce_sum(sumexp[:, j - 1:j],
                                               accs[j - 1][:], axis=X)
                  if nbig:
                      nc.vector.reduce_sum(sumexp[:, nbig - 1:nbig],
                                           accs[nbig - 1][:], axis=X)
              nc.sync.dma_start(out=o_sum[:], in_=sumexp[:])

    nc.compile()
    _drop_dead_table_loads(nc)
    return nc


def _drop_dead_table_loads(nc):
    """Remove ACT_TABLE_LOADs immediately superseded by another load (the
    table-load pass emits a redundant initial set before the real one)."""
    for b in nc.main_func.blocks:
        insts = list(b.instructions)
        dead = []
        pending = None
        for i in insts:
            tn = type(i).__name__
            if 'InstLoadActFuncSet' in tn:
                if pending is not None:
                    dead.append(pending)
                pending = i
            elif 'InstActivation' in tn:
                pending = None
        for i in dead:
            try:
                b.instructions.remove(i)
            except (ValueError, AttributeError):
                pass


def _get_module(repeat=1, skip_cc=False, skip_big=False):
    key = f"nc{repeat}.{skip_cc}.{skip_big}"
    if key not in _STATE:
        _STATE[key] = _build_module(repeat, skip_cc, skip_big)
    return _STATE[key]


def _numpy_fallback(center_id, context_ids, epsilon, emb, prior_means,
                    prior_vars, W_enc, b_enc, W_mean, b_mean, W_var, b_var,
                    W_vocab, b_vocab):
    # Full-precision host computation; only used if b_vocab is nonzero
    # (never happens for this problem's input spec).
    def softplus(x):
        return np.logaddexp(0.0, x)
    ce = emb[center_id]
    cx = emb[context_ids]
    enc_in = np.concatenate(
        [np.broadcast_to(ce[:, None, :], cx.shape), cx], axis=-1)
    h = np.maximum(enc_in @ W_enc + b_enc, 0.0).sum(axis=1)
    mean = h @ W_mean + b_mean
    var = softplus(h @ W_var + b_var)
    z = mean + np.exp(var / 2.0) * epsilon
    logits = z @ W_vocab + b_vocab
    mx = logits.max(axis=1, keepdims=True)
    lse = mx[:, 0] + np.log(np.exp(logits - mx).sum(axis=1))
    logp = logits - lse[:, None]
    pm = prior_means[center_id]
    pv = softplus(prior_vars[center_id])
    dd = pm - mean
    kl = 0.5 * np.sum(var / pv + dd * dd / pv - 1.0
                      + np.log(pv) - np.log(var), axis=1)
    rec = np.take_along_axis(logp, context_ids, axis=1).sum(axis=1)
    return np.float32(np.mean(rec - kl))


def prepare_in_maps(center_id, context_ids, epsilon, emb, prior_means,
                    prior_vars, W_enc, b_enc, W_mean, b_mean, W_var, b_var,
                    W_vocab, b_vocab):
    import ml_dtypes
    center_id = np.asarray(center_id).astype(np.int64)
    context_ids = np.asarray(context_ids).astype(np.int64)
    epsilon = np.asarray(epsilon, dtype=np.float32)
    emb = np.asarray(emb, dtype=np.float32)
    prior_means = np.asarray(prior_means, dtype=np.float32)
    prior_vars = np.asarray(prior_vars, dtype=np.float32)
    W_enc = np.asarray(W_enc, dtype=np.float32)
    b_enc = np.asarray(b_enc, dtype=np.float32)
    W_mean = np.asarray(W_mean, dtype=np.float32)
    b_mean = np.asarray(b_mean, dtype=np.float32)
    W_var = np.asarray(W_var, dtype=np.float32)
    b_var = np.asarray(b_var, dtype=np.float32)
    W_vocab = np.asarray(W_vocab, dtype=np.float32)

    # wpk columns (bf16): we0 | we1 | wm0 | wm1 | wv0 | wv1
    wpk = np.concatenate([
        W_enc[0:D, :], W_enc[D:2 * D, :],
        W_mean[0:D, :], W_mean[D:2 * D, :],
        W_var[0:D, :], W_var[D:2 * D, :]],
        axis=1).astype(ml_dtypes.bfloat16)
    # biases + epsilon ride along in the f32 pps pack
    bp = np.stack([b_enc[:D], b_enc[D:], b_mean, b_var, epsilon],
                  axis=1).astype(np.float32)
    in_maps = []
    for m in range(NCORES):
        s = slice(m * BS, (m + 1) * BS)
        cid = center_id[s]
        ctx = context_ids[s]                      # [BS, C]
        ceT = emb[cid].T                          # [D, BS]
        cxT = emb[ctx].transpose(2, 1, 0).reshape(D, C * BS)
        cxp = np.concatenate([ceT, cxT], axis=1).astype(ml_dtypes.bfloat16)
        pps = np.concatenate([
            prior_means[cid].T, prior_vars[cid].T,
            W_vocab[:, ctx].sum(axis=2), bp], axis=1)  # [D, 3*BS+5]
        wshard = np.zeros((D, VS), dtype=ml_dtypes.bfloat16)
        lo = m * VS
        hi = min((m + 1) * VS, V)
        if hi > lo:
            wshard[:, :hi - lo] = W_vocab[:, lo:hi].astype(ml_dtypes.bfloat16)
        in_maps.append({
            "wpk": np.ascontiguousarray(wpk),
            "cxp": np.ascontiguousarray(cxp),
            "pps": np.ascontiguousarray(pps),
            "wsh": np.ascontiguousarray(wshard),
        })
    return in_maps


def combine_results(results, context_ids, b_vocab):
    """Host combine: results[m] holds o_sum/o_kl/o_rec of core m.

    o_sum[:, j] on core c is the partial exp-sum (over core c's vocab
    shard) for batch block (c + j) % NCORES.
    """
    s_all = np.zeros((NCORES, BS), dtype=np.float64)   # [block, row]
    for c in range(NCORES):
        o = results[c]["o_sum"].astype(np.float64)      # [BS, j]
        for b in range(NCORES):
            s_all[b] += o[:, (b - c) % NCORES]
    SUM = s_all.reshape(B) - float(NPAD) * PAD_EXPVAL
    lse = np.log(SUM)

    kl = np.concatenate(
        [results[m]["o_kl"][0].astype(np.float64) for m in range(NCORES)])
    kl *= 0.5
    rec_pre = np.concatenate(
        [results[m]["o_rec"][0].astype(np.float64) for m in range(NCORES)])
    bsum = b_vocab[context_ids].sum(axis=1).astype(np.float64)
    rec = rec_pre + bsum - C * lse
    return np.float32(np.mean(rec - kl))


def kernel(center_id, context_ids, epsilon, emb, prior_means, prior_vars,
           W_enc, b_enc, W_mean, b_mean, W_var, b_var, W_vocab, b_vocab):
    global LAST_EXEC_TIME_NS, LAST_RESULTS
    center_id = np.asarray(center_id).astype(np.int64)
    context_ids = np.asarray(context_ids).astype(np.int64)
    b_vocab = np.asarray(b_vocab, dtype=np.float32)

    if np.any(b_vocab != 0.0):
        return _numpy_fallback(
            center_id, context_ids,
            np.asarray(epsilon, dtype=np.float32),
            np.asarray(emb, dtype=np.float32),
            np.asarray(prior_means, dtype=np.float32),
            np.asarray(prior_vars, dtype=np.float32),
            np.asarray(W_enc, dtype=np.float32),
            np.asarray(b_enc, dtype=np.float32),
            np.asarray(W_mean, dtype=np.float32),
            np.asarray(b_mean, dtype=np.float32),
            np.asarray(W_var, dtype=np.float32),
            np.asarray(b_var, dtype=np.float32),
            np.asarray(W_vocab, dtype=np.float32), b_vocab)

    from concourse.bass_utils import run_bass_kernel_spmd

    in_maps = prepare_in_maps(center_id, context_ids, epsilon, emb,
                              prior_means, prior_vars, W_enc, b_enc, W_mean,
                              b_mean, W_var, b_var, W_vocab, b_vocab)

    nc = _get_module()
    res = run_bass_kernel_spmd(nc, in_maps, core_ids=list(range(NCORES)))
    LAST_EXEC_TIME_NS = res.exec_time_ns
    LAST_RESULTS = res
    return combine_results(res.results, context_ids, b_vocab)
